# revision 4
# baseline (speedup 1.0000x reference)
"""Trainium2 Bass kernel for nn_CrossAttentionLayer (dual-stream transformer
block), v2.

Sharding: both streams sharded by L across 8 cores (512 rows/stream/core).
Self-attention (seq dim = B=4) is local; cross-attention AllGathers K^T / V
(fp16) per call.  All activations stay in SBUF; activation transposes use the
DMA xbar (f16), which lands rows in natural k-major chunk order; matmuls in
fp16 with fp32 PSUM.  Cross-attention computes transposed scores (keys on
partitions, 2 heads packed in row groups 0-63/64-127) so softmax needs no
partition reductions: exp on ACT, denominator via a ones-column in V,
normalization batched per (b, head-quarter) through reciprocal_approx_fast +
a K=1 PE broadcast.
"""

import numpy as np
from contextlib import ExitStack

import concourse.bass as bass
import concourse.bacc as bacc
import concourse.mybir as mybir
import concourse.tile as tile
from concourse import bass_utils

H = 16
D = 1024
B = 4
HD = 64
EPS = 1e-5
P = 128
KT = D // P          # 8 contraction chunks over D
R = B * P            # 512 rows per stream per core
NC = 8

F16 = mybir.dt.float16
F32 = mybir.dt.float32
AF = mybir.ActivationFunctionType
ALU = mybir.AluOpType
X_AX = mybir.AxisListType.X

# OnT head-slot j = hq*4 + h2*2 + hp2  <->  head h = 4*hq + 2*hp2 + h2
H_OF_J = [4 * (j // 4) + [0, 2, 1, 3][j % 4] for j in range(16)]


def _bcast_part(ap, n):
    return bass.AP(tensor=ap.tensor, offset=ap.offset, ap=[[0, n]] + list(ap.ap))


def _bcast_last(ap, n):
    return bass.AP(tensor=ap.tensor, offset=ap.offset, ap=list(ap.ap) + [[0, n]])


class Em:
    def __init__(self, tc, ctx, io, triv):
        self.tc = tc
        self.nc = tc.nc
        self.io = io
        self.triv = triv
        nc = self.nc

        self.const = ctx.enter_context(tc.tile_pool(name="const", bufs=1))
        # long-lived activation slabs
        self.slab = ctx.enter_context(tc.tile_pool(name="slab", bufs=1))
        # transposed-activation slabs share one 3-deep rotation
        self.tslab = ctx.enter_context(tc.tile_pool(name="tslab", bufs=3))
        # transient working tiles
        self.act = ctx.enter_context(tc.tile_pool(name="act", bufs=1))
        self.ev2 = ctx.enter_context(tc.tile_pool(name="ev2", bufs=2))
        self.ca3 = ctx.enter_context(tc.tile_pool(name="ca3", bufs=3))
        self.w2 = ctx.enter_context(tc.tile_pool(name="w2", bufs=2))
        self.dram = ctx.enter_context(tc.tile_pool(name="dram", bufs=1, space="DRAM"))
        # PSUM: pp (1 bank x2) + pp_s (2 banks x2) + pp_o (1 bank x2) = 8 banks
        self.pp = ctx.enter_context(tc.tile_pool(name="pp", bufs=2, space="PSUM"))
        self.pp_s = ctx.enter_context(tc.tile_pool(name="pp_s", bufs=2, space="PSUM"))
        self.pp_o = ctx.enter_context(tc.tile_pool(name="pp_o", bufs=2, space="PSUM"))

        self.epst = self.const.tile([P, 1], F32)
        nc.vector.memset(self.epst[:], EPS)
        self.ones_col = self.const.tile([P, HD], F16)
        nc.vector.memset(self.ones_col[:], 1.0)

        def rep(name, n):
            t = self.const.tile([P, n], F16)
            nc.gpsimd.dma_start(out=t[:], in_=_bcast_part(io[name], P))
            return t

        self.sa_in_b_rep = None if triv["sa_in_b"] else rep("sa_in_b16", 3 * D)
        self.cv_b_rep = None if triv["cv_b"] else rep("cv_b16", D)
        self.co_b_rep = None if triv["co_b"] else rep("co_b16", D)
        self.b_rep = {}
        for bn in ["b1", "b2", "b3", "b4"]:
            self.b_rep[bn] = None if triv[bn] else rep(bn + "_16", D)
        self.ln_reps = {}
        for lnm in ["n1v", "n2v", "n3v", "n1t", "n2t", "n3t"]:
            if not triv[lnm]:
                self.ln_reps[lnm] = (rep(lnm + "_g16", D), rep(lnm + "_b16", D))
        self.cqbT = self.ckbT = None
        if not triv["cq_b"]:
            self.cqbT = self.const.tile([P, KT], F32)
            nc.sync.dma_start(out=self.cqbT[:], in_=io["cq_bT"])
        if not triv["ck_b"]:
            self.ckbT = self.const.tile([P, KT], F32)
            nc.sync.dma_start(out=self.ckbT[:], in_=io["ck_bT"])

    # ---- weights stream through a 16KB/partition double-buffered slot ----
    def load_w(self, name, third=None):
        """DRAM (D, D) f16 [or a third of (D,3D)] -> SBUF (128, KT, D)."""
        t = self.w2.tile([P, KT, D], F16, tag="w")
        src = self.io[name].rearrange("(k p) n -> p k n", p=P)
        if third is None:
            self.nc.sync.dma_start(out=t[:], in_=src)
        else:
            self.nc.sync.dma_start(out=t[:], in_=src[:, :, third * D:(third + 1) * D])
        return t

    def load_co_half(self, half):
        """coT64 DRAM (64, 16, D) j-half -> SBUF (64, 8, D)."""
        t = self.w2.tile([HD, 8, D], F16, tag="w")
        self.nc.sync.dma_start(
            out=t[:], in_=self.io["coT64"][:, half * 8:(half + 1) * 8, :])
        return t

    # ---- layernorm over free dim of an f32 (128, 1024) tile ----
    def emit_ln(self, r, out, lnm, relu=False):
        nc = self.nc
        st = self.act.tile([P, 2, 6], F32, tag="ln_st")
        rv = r[:].rearrange("p (c n) -> p c n", c=2)
        nc.vector.bn_stats(st[:, 0, :], rv[:, 0, :])
        nc.vector.bn_stats(st[:, 1, :], rv[:, 1, :])
        mv = self.act.tile([P, 2], F32, tag="ln_mv")
        nc.vector.bn_aggr(mv[:], st[:])
        sd = self.act.tile([P, 1], F32, tag="ln_sd")
        nc.scalar.activation(sd[:], mv[:, 1:2], AF.Sqrt, bias=self.epst[:])
        rs = self.act.tile([P, 1], F32, tag="ln_rs")
        nc.vector.reciprocal(rs[:], sd[:])
        nm = self.act.tile([P, 1], F32, tag="ln_nm")
        nc.vector.scalar_tensor_tensor(
            out=nm[:], in0=mv[:, 0:1], scalar=-1.0, in1=rs[:],
            op0=ALU.mult, op1=ALU.mult)
        if self.triv[lnm]:
            nc.scalar.activation(out[:], r[:], AF.Relu if relu else AF.Identity,
                                 bias=nm[:], scale=rs[:])
            return
        y = self.act.tile([P, D], F32, tag="ln_y")
        nc.scalar.activation(y[:], r[:], AF.Identity, bias=nm[:], scale=rs[:])
        g_rep, b_rep = self.ln_reps[lnm]
        nc.vector.tensor_mul(y[:], y[:], g_rep[:])
        nc.vector.tensor_add(y[:], y[:], b_rep[:])
        if relu:
            nc.vector.tensor_scalar_max(out[:], y[:], 0.0)
        else:
            nc.vector.tensor_copy(out[:], y[:])

    # ---- QKV projections for both streams, weight-third at a time ----
    def emit_sa_qkv(self, sa_pool, streams):
        nc = self.nc
        io = self.io
        if not hasattr(self, "xT"):
            self.xT = {}
            self.qkv = {}
        for s in streams:
            xT = self.tslab.tile([P, KT, R], F16, tag="actT", name=f"xT_{s}")
            nc.sync.dma_start(
                out=xT[:], in_=io[f"xT16_{s}"].rearrange("(k p) r -> p k r", p=P))
            self.xT[s] = xT
            self.qkv[s] = [
                sa_pool.tile([P, 3 * D], F16, tag=f"qkv{b}_{s}", name=f"qkv{b}_{s}")
                for b in range(B)]
        for third in range(3):
            sw = self.load_w("sawT16", third=third)
            for s in streams:
                xT = self.xT[s]
                for b in range(B):
                    for n2 in range(2):
                        ps = self.pp.tile([P, 512], F32, tag="proj_ps")
                        for k in range(KT):
                            nc.tensor.matmul(
                                ps[:], lhsT=xT[:, k, b * P:(b + 1) * P],
                                rhs=sw[:, k, n2 * 512:(n2 + 1) * 512],
                                start=(k == 0), stop=(k == KT - 1))
                        off = third * D + n2 * 512
                        if self.sa_in_b_rep is None:
                            nc.scalar.copy(self.qkv[s][b][:, off:off + 512], ps[:])
                        else:
                            nc.vector.tensor_add(
                                self.qkv[s][b][:, off:off + 512], ps[:],
                                self.sa_in_b_rep[:, off:off + 512])

    # ---- self-attention (B=4 seq) + out-proj + LN1 for one stream ----
    def emit_sa_stream(self, s, sa_pool, lnm, sao):
        nc = self.nc
        io = self.io
        qkv = self.qkv[s]

        sc = sa_pool.tile([P, B, H, B], F32, tag=f"sa_sc_{s}")
        for sq in range(B):
            for u in range(B):
                pt = sa_pool.tile([P, D], F16, tag="sa_pt")
                nc.vector.tensor_mul(pt[:], qkv[sq][:, 0:D], qkv[u][:, D:2 * D])
                nc.vector.reduce_sum(
                    out=sc[:, sq, :, u],
                    in_=pt[:].rearrange("p (h d) -> p h d", h=H), axis=X_AX)
        esc = sa_pool.tile([P, B, H, B], F32, tag=f"sa_esc_{s}")
        nc.scalar.activation(esc[:], sc[:], AF.Exp, scale=0.125)
        den = sa_pool.tile([P, B, H], F32, tag="sa_den")
        nc.vector.reduce_sum(out=den[:], in_=esc[:], axis=X_AX)
        rden = sa_pool.tile([P, B, H], F32, tag="sa_rden")
        nc.vector.reciprocal_approx_fast(rden[:], den[:])
        a16 = sa_pool.tile([P, B, H, B], F16, tag=f"sa_a16_{s}")
        nc.vector.tensor_mul(a16[:], esc[:], _bcast_last(rden[:], B))

        x1_16 = self.slab.tile([P, B, D], F16, tag=f"x1_{s}", name=f"x1_{s}")
        x1T = self.tslab.tile([P, KT, R], F16, tag="actT", name=f"x1T_{s}")
        so = sao
        for sq in range(B):
            o = sa_pool.tile([P, D], F16, tag="sa_o")
            ov = o[:].rearrange("p (h d) -> p h d", h=H)
            tmp = sa_pool.tile([P, D], F16, tag="sa_tmp")
            tv = tmp[:].rearrange("p (h d) -> p h d", h=H)
            for u in range(B):
                vv = qkv[u][:, 2 * D:3 * D].rearrange("p (h d) -> p h d", h=H)
                av = _bcast_last(a16[:, sq, :, u], HD)
                if u == 0:
                    nc.vector.tensor_mul(ov, vv, av)
                else:
                    nc.vector.tensor_mul(tv, vv, av)
                    nc.vector.tensor_add(ov, ov, tv)
            oT = sa_pool.tile([P, KT, P], F16, tag="sa_oT")
            nc.sync.dma_start(out=oT[:], in_=o[:], transpose=True)
            rowsb = sa_pool.tile([P, D], F32, tag="rowsb")
            nc.sync.dma_start(out=rowsb[:],
                              in_=io[f"rowsb_{s}"][sq * P:(sq + 1) * P, :])
            rr = sa_pool.tile([P, D], F32, tag="sa_r")
            for nch in range(2):
                ps = self.pp.tile([P, 512], F32, tag="proj_ps")
                for k in range(KT):
                    nc.tensor.matmul(
                        ps[:], lhsT=oT[:, k, :],
                        rhs=so[:, k, nch * 512:(nch + 1) * 512],
                        start=(k == 0), stop=(k == KT - 1))
                sl = slice(nch * 512, (nch + 1) * 512)
                nc.vector.tensor_add(rr[:, sl], ps[:], rowsb[:, sl])
            self.emit_ln(rr, x1_16[:, sq, :], lnm)
            nc.sync.dma_start(out=x1T[:, :, sq * P:(sq + 1) * P],
                              in_=x1_16[:, sq, :], transpose=True)
        return x1_16, x1T

    # ---- feature-major projection into an SBUF slab via staged eviction ----
    def proj_fm(self, xT, wT, biasT, out_sb):
        """out_sb [P, KT, R] f16 chunks = (wT.T x)^T (dout on partitions)."""
        nc = self.nc
        for ot in range(KT):
            ps = self.pp.tile([P, R], F32, tag="proj_ps")
            for k in range(KT):
                nc.tensor.matmul(
                    ps[:], lhsT=wT[:, k, ot * P:(ot + 1) * P], rhs=xT[:, k, :],
                    start=(k == 0), stop=(k == KT - 1))
            if biasT is not None:
                nc.scalar.activation(out_sb[:, ot, :], ps[:], AF.Identity,
                                     bias=biasT[:, ot:ot + 1])
            else:
                nc.scalar.copy(out_sb[:, ot, :], ps[:])

    # ---- K^T and V_ext projections + AllGather ----
    def emit_kv_and_ag(self, xT, which):
        """K/V projections + AllGather, split into head-halves so the first
        pair of collectives lands early and cross-attention (quarters 0-1)
        can start while the second half is still gathering."""
        nc = self.nc
        ck = self.load_w("ckT16")
        cv = self.load_w("cvT16")
        rg = [list(range(NC))]
        kg, vg = [], []
        for j in range(2):
            k_loc = self.dram.tile([D // 2, R], F16, tag=f"kloc{which}{j}")
            kdst = k_loc[:].rearrange("(k p) r -> p k r", p=P)
            for hpl in range(4):
                ot = 4 * j + hpl
                ps = self.pp.tile([P, R], F32, tag="proj_ps")
                for k in range(KT):
                    nc.tensor.matmul(
                        ps[:], lhsT=ck[:, k, ot * P:(ot + 1) * P],
                        rhs=xT[:, k, :],
                        start=(k == 0), stop=(k == KT - 1))
                ev = self.ev2.tile([P, R], F16, tag="kv_ev")
                if self.ckbT is not None:
                    nc.scalar.activation(ev[:], ps[:], AF.Identity,
                                         bias=self.ckbT[:, ot:ot + 1])
                else:
                    nc.scalar.copy(ev[:], ps[:])
                nc.sync.dma_start(out=kdst[:, hpl, :], in_=ev[:])
            k_g = self.dram.tile([NC * (D // 2), R], F16,
                                 tag=f"kg{which}{j}", addr_space="Shared")
            nc.gpsimd.collective_compute(
                "AllGather", ALU.bypass, replica_groups=rg,
                ins=[k_loc[:].opt()], outs=[k_g[:].opt()])
            kg.append(k_g)

            v_loc = self.dram.tile([R, 8 * 65], F16, tag=f"vloc{which}{j}")
            for b in range(B):
                ve = self.ev2.tile([P, 8, 65], F16, tag="v_ev")
                ps = self.pp.tile([P, 512], F32, tag="proj_ps")
                for k in range(KT):
                    nc.tensor.matmul(
                        ps[:], lhsT=xT[:, k, b * P:(b + 1) * P],
                        rhs=cv[:, k, j * 512:(j + 1) * 512],
                        start=(k == 0), stop=(k == KT - 1))
                dst = ve[:, :, 0:HD]
                src = ps[:].rearrange("p (h d) -> p h d", h=8)
                if self.cv_b_rep is None:
                    nc.scalar.copy(dst, src)
                else:
                    nc.vector.tensor_add(
                        dst, src,
                        self.cv_b_rep[:, j * 512:(j + 1) * 512].rearrange(
                            "p (h d) -> p h d", h=8))
                nc.vector.memset(ve[:, :, 64:65], 1.0)
                nc.sync.dma_start(out=v_loc[b * P:(b + 1) * P, :], in_=ve[:])
            v_g = self.dram.tile([NC * R, 8 * 65], F16,
                                 tag=f"vg{which}{j}", addr_space="Shared")
            nc.gpsimd.collective_compute(
                "AllGather", ALU.bypass, replica_groups=rg,
                ins=[v_loc[:].opt()], outs=[v_g[:].opt()])
            vg.append(v_g)
        return kg, vg

    # ---- cross-attention + LN2 for one stream ----
    def emit_ca(self, ca_pool, ca2, qT, kT_g, v_g, x1_16, lnm, s, dbg=None,
                dbgio=None):
        nc = self.nc
        OnT = ca_pool.tile([HD, H, B, P], F16, tag="OnT")
        for hq in range(4):
            jh, hql = divmod(hq, 2)
            kq_src = kT_g[jh][:].rearrange("(c hp p) r -> p c hp r",
                                           c=NC, hp=4)
            vq_src = v_g[jh][:].rearrange("(c x) (h e) -> x c h e",
                                          c=NC, h=8)
            kq = ca2.tile([P, NC, 2, R], F16, tag="kq")
            for i in range(2):
                nc.sync.dma_start(out=kq[:, :, i, :],
                                  in_=kq_src[:, :, 2 * hql + i, :])
            if dbgio is not None and hq == 0:
                nc.sync.dma_start(out=dbgio["dbg_kq"], in_=kq[:])
            for b in range(B):
                vq = ca2.tile([P, NC, 4, 65], F16, tag="vq")
                nc.scalar.dma_start(
                    out=vq[:],
                    in_=vq_src[b * P:(b + 1) * P, :,
                               4 * hql:4 * hql + 4, :])
                if dbgio is not None and hq == 0 and b == 0:
                    nc.sync.dma_start(out=dbgio["dbg_vq"], in_=vq[:])
                pav = self.pp_o.tile([65, 4, P], F32, tag="av_ps")
                # slot = h2*2 + hp2; head h = 4*hq + 2*hp2 + h2 (= H_OF_J)
                for hp2 in range(2):
                    for h2 in range(2):
                        po = h2 * HD
                        slot = h2 * 2 + hp2
                        ps_s = self.pp_s.tile([P, NC, P], F32, tag="s_ps")
                        for c in range(NC):
                            nc.tensor.matmul(
                                ps_s[:, c, :],
                                lhsT=kq[po:po + HD, c, hp2, b * P:(b + 1) * P],
                                rhs=qT[po:po + HD, 2 * hq + hp2,
                                       b * P:(b + 1) * P],
                                start=True, stop=True)
                        aT = self.ca3.tile([P, NC, P], F16, tag="aT")
                        nc.scalar.activation(aT[:], ps_s[:], AF.Exp, scale=0.125)
                        if (dbgio is not None and hq == 0 and b == 0
                                and slot == 0):
                            nc.sync.dma_start(out=dbgio["dbg_aT"], in_=aT[:])
                        for c in range(NC):
                            nc.tensor.matmul(
                                pav[:, slot, :],
                                lhsT=vq[:, c, 2 * hp2 + h2, :],
                                rhs=aT[:, c, :],
                                start=(c == 0), stop=(c == NC - 1))
                # batched normalization of the 4 heads (slot order == j order).
                # Full-range PSUM->SBUF eviction first: partition-offset DVE
                # reads of PSUM (base 64) return garbage on HW (sim accepts
                # them), so all softmax-denominator math runs on the SBUF copy.
                dsb = ca_pool.tile([65, 4, P], F32, tag="dsb")
                nc.vector.tensor_copy(dsb[:], pav[:])
                nc.vector.reciprocal(dsb[64:65, :, :], dsb[64:65, :, :])
                rc16 = ca_pool.tile([65, 4, P], F16, tag="rc16")
                nc.vector.tensor_copy(rc16[64:65, :, :], dsb[64:65, :, :])
                if dbgio is not None and hq == 0 and b == 0:
                    nc.sync.dma_start(out=dbgio["dbg_rc"],
                                      in_=rc16[64:65, :, :])
                pbc = self.pp.tile([HD, 4, P], F32, tag="proj_ps")
                nc.tensor.matmul(pbc[:], lhsT=self.ones_col[64:65, :],
                                 rhs=rc16[64:65, :, :], start=True, stop=True)
                bc16 = ca_pool.tile([HD, 4, P], F16, tag="bc16")
                nc.vector.tensor_copy(bc16[:], pbc[:])
                nc.vector.tensor_mul(OnT[:, 4 * hq:4 * hq + 4, b, :],
                                     dsb[0:HD, :, :], bc16[:])

        if dbg is not None:
            nc.sync.dma_start(out=dbg, in_=OnT[:])
        # out-projection (K=64 per head-slot) + residual + LN2
        coA = self.load_co_half(0)
        coB = self.load_co_half(1)
        x2_16 = self.slab.tile([P, B, D], F16, tag="x2", name=f"x2_{s}")
        x2T = self.tslab.tile([P, KT, R], F16, tag="actT", name=f"x2T_{s}")
        for b in range(B):
            rr = ca_pool.tile([P, D], F32, tag="ca_r")
            for nch in range(2):
                ps = self.pp.tile([P, 512], F32, tag="proj_ps")
                for j in range(H):
                    co = coA if j < 8 else coB
                    nc.tensor.matmul(
                        ps[:], lhsT=OnT[:, j, b, :],
                        rhs=co[:, j % 8, nch * 512:(nch + 1) * 512],
                        start=(j == 0), stop=(j == H - 1))
                sl = slice(nch * 512, (nch + 1) * 512)
                nc.vector.tensor_add(rr[:, sl], ps[:], x1_16[:, b, sl])
            if self.co_b_rep is not None:
                nc.vector.tensor_add(rr[:], rr[:], self.co_b_rep[:])
            self.emit_ln(rr, x2_16[:, b, :], lnm)
            nc.sync.dma_start(out=x2T[:, :, b * P:(b + 1) * P],
                              in_=x2_16[:, b, :], transpose=True)
        return x2_16, x2T

    # ---- FFN ----
    def emit_ffn(self, ffn_pool, xT, x_16, w1n, w2n, b1n, b2n, lnm, out_dram):
        nc = self.nc
        w1 = self.load_w(w1n)
        w2 = self.load_w(w2n)
        b1_rep = self.b_rep[b1n]
        b2_rep = self.b_rep[b2n]
        for b in range(B):
            h1 = ffn_pool.tile([P, D], F32, tag="ffn_h1")
            for nch in range(2):
                ps = self.pp.tile([P, 512], F32, tag="proj_ps")
                for k in range(KT):
                    nc.tensor.matmul(
                        ps[:], lhsT=xT[:, k, b * P:(b + 1) * P],
                        rhs=w1[:, k, nch * 512:(nch + 1) * 512],
                        start=(k == 0), stop=(k == KT - 1))
                sl = slice(nch * 512, (nch + 1) * 512)
                if b1_rep is None:
                    nc.vector.tensor_copy(h1[:, sl], ps[:])
                else:
                    nc.vector.tensor_add(h1[:, sl], ps[:], b1_rep[:, sl])
            hr = ffn_pool.tile([P, D], F16, tag="ffn_hr")
            self.emit_ln(h1, hr, lnm, relu=True)
            hT = ffn_pool.tile([P, KT, P], F16, tag="ffn_hT")
            nc.sync.dma_start(out=hT[:], in_=hr[:], transpose=True)
            oo = ffn_pool.tile([P, D], F32, tag="ffn_oo")
            for nch in range(2):
                ps = self.pp.tile([P, 512], F32, tag="proj_ps")
                for k in range(KT):
                    nc.tensor.matmul(
                        ps[:], lhsT=hT[:, k, :],
                        rhs=w2[:, k, nch * 512:(nch + 1) * 512],
                        start=(k == 0), stop=(k == KT - 1))
                sl = slice(nch * 512, (nch + 1) * 512)
                nc.vector.tensor_add(oo[:, sl], ps[:], x_16[:, b, sl])
                if b2_rep is not None:
                    nc.vector.tensor_add(oo[:, sl], oo[:, sl], b2_rep[:, sl])
            nc.sync.dma_start(out=out_dram[b, :, :], in_=oo[:])


def emit_full(tc, io, triv):
    with ExitStack() as ctx:
        em = Em(tc, ctx, io, triv)

        with tc.tile_pool(name="sa_pool", bufs=1) as sa_pool:
            em.emit_sa_qkv(sa_pool, ["text"])
            sao_t = em.load_w("saoT16")
            text1, text1T = em.emit_sa_stream("text", sa_pool, "n1t", sao_t)
            # issue the first AllGather as early as possible; everything
            # below up to emit_ca (img QKV + SA + both Q projections)
            # overlaps it
            k1g, v1g = em.emit_kv_and_ag(text1T, 1)
            cq_t = em.load_w("cqT16")
            qT2 = em.slab.tile([P, KT, R], F16, tag="qT2")
            em.proj_fm(text1T, cq_t, em.cqbT, qT2)
            em.emit_sa_qkv(sa_pool, ["img"])
            sao_i = em.load_w("saoT16")
            img1, img1T = em.emit_sa_stream("img", sa_pool, "n1v", sao_i)
            cq_i = em.load_w("cqT16")
            qT1 = em.slab.tile([P, KT, R], F16, tag="qT1")
            em.proj_fm(img1T, cq_i, em.cqbT, qT1)

        with tc.tile_pool(name="ca_pool", bufs=1) as ca_pool, \
             tc.tile_pool(name="ca2", bufs=2) as ca2, \
             tc.tile_pool(name="ffn_pool", bufs=1) as ffn_pool:
            img2, img2T = em.emit_ca(ca_pool, ca2, qT1, k1g, v1g, img1, "n2v",
                                     "img", dbg=io.get("dbg_OnT"),
                                     dbgio=io if DEBUG else None)
            if DEBUG:
                em.nc.sync.dma_start(out=io["dbg_x2i"], in_=img2[:])
            k2g, v2g = em.emit_kv_and_ag(img2T, 2)
            em.emit_ffn(ffn_pool, img2T, img2, "w1T16", "w2T16", "b1", "b2",
                        "n3v", io["img_out"])
            text2, text2T = em.emit_ca(ca_pool, ca2, qT2, k2g, v2g, text1,
                                       "n2t", "text")
            em.emit_ffn(ffn_pool, text2T, text2, "w3T16", "w4T16", "b3", "b4",
                        "n3t", io["text_out"])


# ======================= host side =======================

def _triviality(inputs):
    t = {}
    for lnm in ["n1v", "n2v", "n3v", "n1t", "n2t", "n3t"]:
        t[lnm] = bool(np.all(inputs[lnm + "_g"] == 1.0) and
                      np.all(inputs[lnm + "_b"] == 0.0))
    for bn in ["sa_in_b", "cv_b", "co_b", "cq_b", "ck_b", "b1", "b2", "b3", "b4"]:
        t[bn] = bool(np.all(inputs[bn] == 0.0))
    return t


def _host_prep(inputs, NCC):
    f16, f32 = np.float16, np.float32
    c = {}
    c["sawT16"] = np.ascontiguousarray(inputs["sa_in_w"].T.astype(f16))
    c["saoT16"] = np.ascontiguousarray(inputs["sa_out_w"].T.astype(f16))
    c["cqT16"] = np.ascontiguousarray(inputs["cq_w"].T.astype(f16))
    c["ckT16"] = np.ascontiguousarray(inputs["ck_w"].T.astype(f16))
    c["cvT16"] = np.ascontiguousarray(inputs["cv_w"].T.astype(f16))
    coT = inputs["co_w"].T.astype(f16).reshape(H, HD, D)   # (h, hd, dout)
    c["coT64"] = np.ascontiguousarray(coT[H_OF_J].transpose(1, 0, 2))
    for w in ["w1", "w2", "w3", "w4"]:
        c[w + "T16"] = np.ascontiguousarray(inputs[w].T.astype(f16))
    c["sa_in_b16"] = inputs["sa_in_b"].astype(f16)
    for bn in ["cv_b", "co_b"]:
        c[bn + "16"] = inputs[bn].astype(f16)
    for bn in ["b1", "b2", "b3", "b4"]:
        c[bn + "_16"] = inputs[bn].astype(f16)
    c["cq_bT"] = np.ascontiguousarray(inputs["cq_b"].astype(f32).reshape(KT, P).T)
    c["ck_bT"] = np.ascontiguousarray(inputs["ck_b"].astype(f32).reshape(KT, P).T)
    for lnm in ["n1v", "n2v", "n3v", "n1t", "n2t", "n3t"]:
        c[lnm + "_g16"] = inputs[lnm + "_g"].astype(f16)
        c[lnm + "_b16"] = inputs[lnm + "_b"].astype(f16)

    sa_out_b = inputs["sa_out_b"].astype(f32)
    per_core = []
    L = inputs["img_input"].shape[1]
    TC = L // NCC
    for ci in range(NCC):
        m = {}
        for s, key in [("img", "img_input"), ("text", "text_input")]:
            x = np.asarray(inputs[key][:, ci * TC:(ci + 1) * TC, :], dtype=f32)
            rows = x.reshape(B * TC, D)
            m[f"xT16_{s}"] = np.ascontiguousarray(rows.T.astype(f16))
            m[f"rowsb_{s}"] = np.ascontiguousarray(rows + sa_out_b)
        per_core.append(m)
    return c, per_core


def make_nc(NCC, triv):
    nc = bacc.Bacc("TRN2", target_bir_lowering=False, debug=False,
                   num_devices=NCC)
    io = {}

    def din(name, shape, dt):
        io[name] = nc.dram_tensor(name, list(shape), dt, kind="ExternalInput").ap()

    din("sawT16", (D, 3 * D), F16)
    din("saoT16", (D, D), F16)
    for nm in ["cqT16", "ckT16", "cvT16", "w1T16", "w2T16", "w3T16", "w4T16"]:
        din(nm, (D, D), F16)
    din("coT64", (HD, H, D), F16)
    din("sa_in_b16", (3 * D,), F16)
    for nm in ["cv_b16", "co_b16", "b1_16", "b2_16", "b3_16", "b4_16"]:
        din(nm, (D,), F16)
    din("cq_bT", (P, KT), F32)
    din("ck_bT", (P, KT), F32)
    for lnm in ["n1v", "n2v", "n3v", "n1t", "n2t", "n3t"]:
        din(lnm + "_g16", (D,), F16)
        din(lnm + "_b16", (D,), F16)
    for s in ["img", "text"]:
        din(f"xT16_{s}", (D, R), F16)
        din(f"rowsb_{s}", (R, D), F32)
    io["img_out"] = nc.dram_tensor("img_out", [B, P, D], F32,
                                   kind="ExternalOutput").ap()
    io["text_out"] = nc.dram_tensor("text_out", [B, P, D], F32,
                                    kind="ExternalOutput").ap()
    if DEBUG:
        io["dbg_x1t"] = nc.dram_tensor("dbg_x1t", [P, B, D], F16,
                                       kind="ExternalOutput").ap()
        io["dbg_x1Tt"] = nc.dram_tensor("dbg_x1Tt", [P, KT, R], F16,
                                        kind="ExternalOutput").ap()
        io["dbg_qT2"] = nc.dram_tensor("dbg_qT2", [P, KT, R], F16,
                                       kind="ExternalOutput").ap()
        io["dbg_OnT"] = nc.dram_tensor("dbg_OnT", [HD, H, B, P], F16,
                                       kind="ExternalOutput").ap()
        io["dbg_x2i"] = nc.dram_tensor("dbg_x2i", [P, B, D], F16,
                                       kind="ExternalOutput").ap()
        io["dbg_kg"] = nc.dram_tensor("dbg_kg", [NC * D, R], F16,
                                      kind="ExternalOutput").ap()
        io["dbg_vg"] = nc.dram_tensor("dbg_vg", [NC * R, H * 65], F16,
                                      kind="ExternalOutput").ap()
        io["dbg_kq"] = nc.dram_tensor("dbg_kq", [P, NC, 2, R], F16,
                                      kind="ExternalOutput").ap()
        io["dbg_vq"] = nc.dram_tensor("dbg_vq", [P, NC, 4, 65], F16,
                                      kind="ExternalOutput").ap()
        io["dbg_aT"] = nc.dram_tensor("dbg_aT", [P, NC, P], F16,
                                      kind="ExternalOutput").ap()
        io["dbg_rc"] = nc.dram_tensor("dbg_rc", [1, 4, P], F16,
                                      kind="ExternalOutput").ap()

    with tile.TileContext(nc, num_cores=NCC) as tc:
        emit_full(tc, io, triv)
    nc.finalize()
    return nc


DEBUG = False

LAST_RESULT = None


def kernel(**inputs):
    NCC = 8
    inputs = {k: np.asarray(v) for k, v in inputs.items()}
    L = inputs["img_input"].shape[1]
    TC = L // NCC
    triv = _triviality(inputs)
    nc = make_nc(NCC, triv)
    common, per_core = _host_prep(inputs, NCC)
    in_maps = [dict(common, **pc) for pc in per_core]
    res = bass_utils.run_bass_kernel_spmd(nc, in_maps, core_ids=list(range(NCC)))
    global LAST_RESULT
    LAST_RESULT = res
    img = np.empty((B, L, D), np.float32)
    text = np.empty((B, L, D), np.float32)
    for ci in range(NCC):
        img[:, ci * TC:(ci + 1) * TC, :] = res.results[ci]["img_out"]
        text[:, ci * TC:(ci + 1) * TC, :] = res.results[ci]["text_out"]
    return img, text


if __name__ == "__main__":
    # Local functional validation in MultiCoreSim (no hardware).
    import sys as _sys
    _sys.path.insert(0, "/root/problem")
    import reference
    from concourse.bass_interp import MultiCoreSim

    inputs = {k: np.asarray(v) for k, v in reference.setup_inputs().items()}
    exp_img, exp_text = [np.asarray(x) for x in reference.reference(**inputs)]
    triv = _triviality(inputs)
    nc = make_nc(NC, triv)
    common, per_core = _host_prep(inputs, NC)
    sim = MultiCoreSim(nc, num_cores=NC, require_finite=True, require_nnan=True)
    cores = list(sim.cores.values())
    for ci, cs in enumerate(cores):
        for k, v in {**common, **per_core[ci]}.items():
            cs.tensor(k)[:] = v
    sim.simulate(check_with_hw=False)
    L = inputs["img_input"].shape[1]
    TC = L // NC
    img = np.empty((B, L, D), np.float32)
    text = np.empty((B, L, D), np.float32)
    for ci, cs in enumerate(cores):
        img[:, ci * TC:(ci + 1) * TC, :] = cs.tensor("img_out")
        text[:, ci * TC:(ci + 1) * TC, :] = cs.tensor("text_out")

    def rel_err(a, e):
        a = np.asarray(a, np.float64)
        e = np.asarray(e, np.float64)
        return float(np.linalg.norm(a - e) / (np.linalg.norm(e) + 1e-30))

    ei, et = rel_err(img, exp_img), rel_err(text, exp_text)
    print(f"SIM relative error: img {ei:.3e}, text {et:.3e}")
